# revision 36
# baseline (speedup 1.0000x reference)
"""ALayer kernel for 8 TRN2 NeuronCores — pure data parallel over batch.

Per-core shard: 4 images of [256, 56, 56].
  h  = relu(conv3x3(x_in, w1))      # 256 -> 16 ch
  A  = sigmoid(conv3x3(h, w2))      # 16 -> 1 ch
  out = x_out * box3x3(A)           # broadcast over 256 ch

v8 design — stall-free software pipeline against the DMA roofline
(HBM ~16.3MB/core ≈ 46us floor):
  conv1: v6's column-tiled rounds (4 concurrent 32-col strips, fp8,
         M=16, N=392; 18 rounds x 2 supergroups).  Relu evacs write the
         h plane directly (2 ACT + 2 DVE per supergroup).
  conv2: direct 9-tap col-tiled rounds (K=16, M=1) — no im2col copies;
         keeps the PE stream free of DMA-latency head-blocks.
  a9:    v6 scatter construction (6 scatters + 2 row-shifts, SWDGE/Q7).
  box:   7 blocks of 8 rows: K=9 ones matmul -> psum [128,8,56];
         evacs to bf16 `ab` split ACT/DVE; muls are bf16 tensor_tensor
         (DVE 2x mode) in [128,2,1792]/[128,2,1344] chunks -> ot.
  Schedule: PE stream is one dense FIFO: conv1(i) rounds interleave
         box(i-2) and conv2(i-1) at ratios that match each dep's
         latency (relu-evac ~1.5us, sigmoid+a9 ~3us).  Loads are paced
         by pool-reuse WAR deps (xpad bufs=2, xo bufs=3) so the SDMA
         engines never starve the a9 scatters.  Loads ride the SP
         HWDGE ring; stores ride the ACT ring; a9 is SWDGE on Q7.
         Short fp8 warm matmuls cover the HAM ramp and tail dep gaps.
"""

import numpy as np
import ml_dtypes

import concourse.bass as bass
import concourse.tile as tile
import concourse.mybir as mybir
from concourse import bacc
from concourse.bass_utils import run_bass_kernel_spmd

BF16 = mybir.dt.bfloat16
FP8 = mybir.dt.float8e4
F32 = mybir.dt.float32

B, C, H, W = 32, 256, 56, 56
NCORES = 8
BL = B // NCORES          # images per core
KCH = 2                   # 256 = 2 chunks of 128
HP = H + 2                # padded plane side (58)
HW = H * W                # 3136
PL = HP * HP              # 3364

_cache = {}


def _build():
    nc = bacc.Bacc("TRN2", target_bir_lowering=False, debug=False)

    xin_d = nc.dram_tensor("xin", [BL, KCH, 128, PL], FP8, kind="ExternalInput").ap()
    xout_d = nc.dram_tensor("xout", [BL, 128, KCH, HW], BF16, kind="ExternalInput").ap()
    w1_d = nc.dram_tensor("w1t", [128, KCH, 9, 16], FP8, kind="ExternalInput").ap()
    w2_d = nc.dram_tensor("w2t", [128, 9], BF16, kind="ExternalInput").ap()
    out_d = nc.dram_tensor("out", [BL, 128, KCH, HW], BF16, kind="ExternalOutput").ap()

    with tile.TileContext(nc) as tc:
        with (
            tc.tile_pool(name="const", bufs=1) as constp,
            tc.tile_pool(name="xpad", bufs=4) as xpadp,
            tc.tile_pool(name="h1", bufs=2) as h1p,
            tc.tile_pool(name="at", bufs=2) as atp,
            tc.tile_pool(name="a9", bufs=2) as a9p,
            tc.tile_pool(name="ab", bufs=2) as abp,
            tc.tile_pool(name="xo", bufs=2) as xop,
            tc.tile_pool(name="ot", bufs=2) as otp,
            tc.tile_pool(name="ps_h", bufs=2, space="PSUM") as ps_h,
            tc.tile_pool(name="ps_a", bufs=2, space="PSUM") as ps_a,
            tc.tile_pool(name="ps_b", bufs=3, space="PSUM") as ps_b,
            tc.tile_pool(name="ps_w", bufs=1, space="PSUM") as ps_w,
        ):
            w1sb = constp.tile([128, KCH, 9, 16], FP8)
            w2sb = constp.tile([128, 9], BF16)
            # full-K ones: rows 0-8 = 1, rows 9-127 = 0.  K=128 matmuls
            # (with zero-padded weights/data) cost the same N cycles but
            # keep the PE array's activity monitor (HAM) seeing a busy
            # array, so the clock stays at 8/8.
            ones9 = constp.tile([128, 128], BF16)
            wl = constp.tile([128, 128], FP8)
            wr = constp.tile([128, 512], FP8)

            # ---- HBM loads on the SP ring; pool-reuse WAR deps pace them ----
            xpads, xos = [], []
            for img in range(BL):
                xpads.append(
                    xpadp.tile([128, KCH, HP, HP], FP8, name="xpad")
                )
                xos.append(xop.tile([128, KCH, HW], BF16, name="xo"))
            MID = 30 * HP

            def load_xin(img, split):
                xpf = xpads[img].rearrange("p k r w -> p k (r w)")
                if split:
                    for k in range(KCH):
                        nc.sync.dma_start(xpf[:, k, 0:MID], xin_d[img, k, :, 0:MID])
                    for k in range(KCH):
                        nc.sync.dma_start(xpf[:, k, MID:PL], xin_d[img, k, :, MID:PL])
                else:
                    for k in range(KCH):
                        nc.sync.dma_start(xpf[:, k, :], xin_d[img, k, :, :])

            xpf0 = xpads[0].rearrange("p k r w -> p k (r w)")
            nc.sync.dma_start(xpf0[:, 0, 0:MID], xin_d[0, 0, :, 0:MID])
            nc.sync.dma_start(w1sb[:], w1_d[:])
            nc.sync.dma_start(w2sb[:], w2_d[:])
            nc.sync.dma_start(xpf0[:, 1, 0:MID], xin_d[0, 1, :, 0:MID])
            nc.sync.dma_start(xpf0[:, 0, MID:PL], xin_d[0, 0, :, MID:PL])
            nc.sync.dma_start(xpf0[:, 1, MID:PL], xin_d[0, 1, :, MID:PL])
            # order + xo-pool WAR gates (bufs=2) stretch the load stream
            # so the SDMA engines have idle windows to serve the small
            # latency-critical SWDGE a9 scatters quickly.  Real deadlines:
            # xin(i) by conv1(i), xo(i) only by box(i)'s muls.
            load_xin(1, True)
            nc.sync.dma_start(xos[0][:], xout_d[0])
            load_xin(2, False)
            nc.sync.dma_start(xos[1][:], xout_d[1])
            load_xin(3, False)
            nc.sync.dma_start(xos[2][:], xout_d[2])
            nc.sync.dma_start(xos[3][:], xout_d[3])

            # ---- warm-up / filler matmuls (no data deps) ----
            nc.vector.memset(ones9[:], 0.0)
            nc.vector.memset(ones9[0:9, :], 1.0)
            nc.gpsimd.memset(wl[:], 0.0)
            nc.gpsimd.memset(wr[:], 0.0)

            def warm(n):
                for _ in range(n):
                    wp = ps_w.tile([128, 512], F32)
                    nc.tensor.matmul(
                        wp[:], wl[:], wr[:],
                        start=True, stop=True, skip_group_check=True,
                    )

            # pre-create plane tiles; zero guards + K-pad partitions for
            # both pool buffers NOW, while ACT/DVE idle during the loads
            h1s = [h1p.tile([128, HP, HP], BF16, name="h1") for _ in range(BL)]
            ats = [atp.tile([128, 2, 7, HP], BF16, name="at") for _ in range(BL)]
            a9s = [a9p.tile([128, HP, HP], BF16, name="a9") for _ in range(BL)]
            for img in range(2):
                h1, at, a9 = h1s[img], ats[img], a9s[img]
                nc.scalar.memzero(h1[:, :, :])
                nc.vector.memset(a9[:, :, :], 0.0)
                nc.vector.memset(at[:, :, :, 0], 0.0)
                nc.vector.memset(at[:, :, :, 57], 0.0)

            def gen_conv1(img):
                """36 PE rounds; relu evacs into the h plane on round 18/36."""
                xpad = xpads[img]
                h1 = h1s[img]
                for s in range(2):
                    ps = ps_h.tile([128, 7, 56], F32)
                    rnd = 0
                    for k in range(KCH):
                        for t in range(9):
                            dy, dx = t // 3, t % 3
                            for j in range(4):
                                rs = 28 * s + j + dy
                                nc.tensor.matmul(
                                    ps[32 * j : 32 * j + 16],
                                    w1sb[:, k, t, :],
                                    xpad[:, k, rs : rs + 25 : 4, dx : dx + 56],
                                    start=(rnd == 0),
                                    stop=(rnd == 17),
                                    tile_position=(0, 32 * j),
                                    skip_group_check=True,
                                )
                            rnd += 1
                            if rnd == 18:
                                for j in range(4):
                                    r0 = 1 + 28 * s + j
                                    dst = h1[0:16, r0 : r0 + 25 : 4, 1:57]
                                    if j < 2:
                                        nc.scalar.activation(
                                            dst, ps[32 * j : 32 * j + 16],
                                            mybir.ActivationFunctionType.Relu,
                                        )
                                    else:
                                        nc.vector.tensor_scalar_max(
                                            dst, ps[32 * j : 32 * j + 16], 0.0
                                        )
                            yield

            def gen_conv2(img):
                """18 PE rounds (9 taps x 2 supergroups); sigmoid; a9 build."""
                h1 = h1s[img]
                at = ats[img]
                a9 = a9s[img]
                a9f = a9.rearrange("p r w -> p (r w)")
                for s in range(2):
                    ps = ps_a.tile([128, 7, 56], F32)
                    rnd = 0
                    for dy in range(3):
                        for dx in range(3):
                            for j in range(4):
                                b = 4 * s + j
                                nc.tensor.matmul(
                                    ps[32 * j : 32 * j + 1],
                                    w2sb[:, rnd : rnd + 1],
                                    h1[:, 7 * b + dy : 7 * b + dy + 7, dx : dx + 56],
                                    start=(rnd == 0),
                                    stop=(rnd == 8),
                                    tile_position=(0, 32 * j),
                                    skip_group_check=True,
                                )
                            rnd += 1
                            if rnd == 9:
                                nc.scalar.activation(
                                    at[:, s, :, 1:57], ps[:],
                                    mybir.ActivationFunctionType.Sigmoid,
                                )
                                # scatter this supergroup's rows right away
                                for c in range(3):
                                    st = (1 + 28 * s) * HP + (1 - c)
                                    nc.gpsimd.dma_start(
                                        a9f[3 + c : 4 + c, st : st + 1624],
                                        at[0:128:32, s],
                                    )
                                if s == 1:
                                    nc.gpsimd.dma_start(
                                        a9f[0:3, HP : 57 * HP],
                                        a9f[3:6, 0 : 56 * HP],
                                    )
                                    nc.gpsimd.dma_start(
                                        a9f[6:9, HP : 57 * HP],
                                        a9f[3:6, 2 * HP : PL],
                                    )
                            yield

            def gen_box(img):
                """7 blocks of 8 rows; evacs split ACT/DVE; bf16 muls."""
                a9 = a9s[img]
                xo = xos[img]
                ab = abp.tile([128, 56, 56], BF16)
                abf = ab.rearrange("p r w -> p (r w)")
                ot = otp.tile([128, KCH, HW], BF16)

                def halfdone(h):
                    # per-k mul chunks stay under ~1us so they never
                    # head-of-line-block the latency-critical relu evacs
                    # behind them in the DVE FIFO
                    s0, s1 = (0, 1792) if h == 0 else (1792, HW)
                    for k in range(KCH):
                        nc.vector.tensor_mul(
                            ot[:, k, s0:s1],
                            xo[:, k, s0:s1],
                            abf[:, s0:s1],
                        )
                    nc.scalar.dma_start(
                        out_d[img, :, :, s0:s1], ot[:, :, s0:s1]
                    )

                for R in range(7):
                    psb = ps_b.tile([128, 8, 56], F32)
                    nc.tensor.matmul(
                        psb[:], ones9[:],
                        a9[:, 1 + 8 * R : 9 + 8 * R, 1:57],
                        start=True, stop=True,
                    )
                    dst = ab[:, 8 * R : 8 * R + 8, :]
                    if R % 2 == 0:
                        nc.scalar.activation(
                            dst, psb[:], mybir.ActivationFunctionType.Copy
                        )
                    else:
                        nc.vector.tensor_copy(dst, psb[:])
                    if R == 3:
                        halfdone(0)
                    yield
                halfdone(1)          # second half pulled as an 8th next()
                yield

            def run(gen, n):
                for _ in range(n):
                    next(gen, None)

            c1 = [gen_conv1(i) for i in range(BL)]
            c2 = [gen_conv2(i) for i in range(BL)]
            bx = [gen_box(i) for i in range(BL)]

            def block(i):
                # conv1(i): r0-5 solo, r6-17 with conv2(i-1) (3:2) —
                # conv2(i-1) and BOTH its sigmoids are fully emitted
                # before conv1 round 18 emits the relu evacs, so the
                # sigmoids never queue behind a relu that depends on
                # later PE work (ACT FIFO head-of-line).  r18-35 carry
                # box(i-2) (1:2).  box(i-3)'s deferred second-half muls
                # are pulled first, AFTER relu(i-1)'s evacs entered the
                # DVE FIFO at the end of the previous block.
                if i >= 3:
                    run(bx[i - 3], 1)
                run(c1[i], 6)
                for _ in range(6):
                    run(c2[i - 1], 3)
                    run(c1[i], 2)
                run(c1[i], 4)
                for _ in range(7):
                    run(bx[i - 2], 1)
                    run(c1[i], 2)

            warm(3)
            run(c1[0], 36)
            run(c1[1], 6)
            for _ in range(6):       # conv2(0) over conv1(1) r6..17
                run(c2[0], 3)
                run(c1[1], 2)
            run(c1[1], 18)           # a9(0) chain completes here
            block(2)                 # conv1(2) + conv2(1) + box(0)
            block(3)                 # conv1(3) + conv2(2) + box(1)
            run(bx[1], 1)            # box(1) second-half muls + store
            warm(4)                  # h(3) evac cover
            for _ in range(6):       # conv2(3) + box(2)
                run(c2[3], 3)
                run(bx[2], 1)
            run(bx[2], 1)
            run(bx[2], 1)
            warm(14)                 # a9(3) chain cover
            run(bx[3], 8)

    nc.compile()
    return nc


def _prep_shards(x_in, x_out, w1, w2):
    bf16 = ml_dtypes.bfloat16
    fp8 = ml_dtypes.float8_e4m3
    # w1t[c, k, t, m] = w1[m, 128k + c, dy, dx],  t = 3*dy + dx
    w1t = np.ascontiguousarray(
        w1.reshape(16, KCH, 128, 9).transpose(2, 1, 3, 0)
    ).astype(fp8)
    # w2t[m, t] = w2[0, m, dy, dx]
    w2t = np.zeros((128, 9), dtype=bf16)
    w2t[0:16, :] = w2[0].reshape(16, 9).astype(bf16)
    xi = np.zeros((NCORES, BL, KCH, 128, HP, HP), dtype=fp8)
    xi[..., 1 : 1 + H, 1 : 1 + W] = (
        x_in.reshape(NCORES, BL, KCH, 128, H, W).astype(fp8)
    )
    xi = xi.reshape(NCORES, BL, KCH, 128, PL)
    # xout[img, c_partition, k, hw]
    xo = np.ascontiguousarray(
        x_out.reshape(NCORES, BL, KCH, 128, HW).transpose(0, 1, 3, 2, 4)
    ).astype(bf16)
    return [
        {
            "xin": np.ascontiguousarray(xi[i]),
            "xout": xo[i],
            "w1t": w1t,
            "w2t": w2t,
        }
        for i in range(NCORES)
    ]


def _run(in_maps, trace=False):
    if "nc" not in _cache:
        _cache["nc"] = _build()
    return run_bass_kernel_spmd(
        _cache["nc"], in_maps, core_ids=list(range(NCORES)), trace=trace
    )


def kernel(x_in, x_out, w1, w2, _trace=False):
    in_maps = _prep_shards(
        np.asarray(x_in, dtype=np.float32),
        np.asarray(x_out, dtype=np.float32),
        np.asarray(w1, dtype=np.float32),
        np.asarray(w2, dtype=np.float32),
    )
    res = _run(in_maps, trace=_trace)
    # out[img, c_partition, k, hw] bf16 -> [B, C, H, W] fp32
    out = np.stack([res.results[i]["out"] for i in range(NCORES)])
    kernel.last_exec_time_ns = res.exec_time_ns
    out = out.astype(np.float32).transpose(0, 1, 3, 2, 4)
    return out.reshape(B, C, H, W)
